# revision 2
# baseline (speedup 1.0000x reference)
"""CrossModalAttention on 8 Trainium2 NeuronCores (Bass/Tile, SPMD).

Sharding: data-parallel over batch B=8 (one batch element per core),
weights replicated. Each core computes, for its batch element:

  fp_i = relu(f_i @ Wp_i^T + bp_i)            i in {1,2,3}
  q_i, k_i = fp_i @ Wq_i^T, fp_i @ Wk_i^T ;  v_i = fp_i @ Wv_i^T
  s1 = (q2+q3) k1^T ; s2 = q1 k3^T + q3 k2^T ; s3 = (q1+q2) k3^T
  u_i = softmax(s_i) v_i
  out = concat([u1, u2, u3, fp1, fp2, fp3], -1)

v2 design: matmuls in bf16 except the p@V attention matmul, which runs
fp8e4m3 DoubleRow (2x PE rate; p and v quantized to fp8 after bf16-
accurate computation — measured rel err 1.57e-2 vs the 2e-2 gate).
Every intermediate (fpT, qT, kT, v) is SBUF-resident — zero DRAM
spills.  Scores accumulate the two-product sums directly in PSUM
(no c13 staging).  softmax keeps p unnormalized; the 1/denom scale is
applied to u after the AV matmul.  Outputs leave the device in bf16,
fp_i transposed ([D,T]); the host converts to fp32 and restores the
reference layout.  p-transposes are interleaved between AV matmuls to
keep the PE stream gapless.
"""

import math
from contextlib import ExitStack

import numpy as np

FP = None  # set in _lazy_imports
_STATE = {}

B, T, D = 8, 1024, 1024
P = 128
Cd = D // P
Ct = T // P
S = 512           # matmul free-dim tile (one PSUM bank of fp32)
WCH = 512         # weight streaming chunk (output columns per load)
AV_FP8 = True     # fp8e4m3 DoubleRow for the V projection and p@V matmuls


def _lazy_imports():
    global tile, bacc, mybir, bass, make_identity, FP, BF, F8, AF, DR
    import concourse.bass as bass
    import concourse.tile as tile
    from concourse import bacc, mybir
    from concourse.masks import make_identity
    FP = mybir.dt.float32
    BF = mybir.dt.bfloat16
    F8 = mybir.dt.float8e4
    AF = mybir.ActivationFunctionType
    DR = mybir.MatmulPerfMode.DoubleRow


def build_nc(reps=1):
    _lazy_imports()

    nc = bacc.Bacc("TRN2", target_bir_lowering=False, debug=False,
                   enable_asserts=False, num_devices=8)

    fT, Wr, bp = [], {}, []
    for i in range(3):
        fT.append(nc.dram_tensor(f"fT{i}", [D, T], BF, kind="ExternalInput").ap())
        for nm in ("p", "q", "k", "v"):
            Wr[(nm, i)] = nc.dram_tensor(
                f"W{nm}r{i}", [P, Cd * D], BF, kind="ExternalInput").ap()
        bp.append(nc.dram_tensor(f"bp{i}", [D], FP, kind="ExternalInput").ap())
    out_u = nc.dram_tensor("out_u", [T, 3 * D], BF, kind="ExternalOutput").ap()
    out_fT = nc.dram_tensor("out_fT", [3 * D, T], BF, kind="ExternalOutput").ap()

    with tile.TileContext(nc) as tc:
        with ExitStack() as top:
            const = top.enter_context(tc.tile_pool(name="const", bufs=1))
            ident_f = const.tile([P, P], FP)
            make_identity(nc, ident_f[:])
            ident = const.tile([P, P], BF)
            nc.vector.tensor_copy(ident[:], ident_f[:])
            bias_t = const.tile([P, 3 * Cd], FP)
            for i in range(3):
                nc.sync.dma_start(bias_t[:, i * Cd:(i + 1) * Cd],
                                  bp[i].rearrange("(c p) -> p c", p=P))
            for _rep in range(reps):
                _emit_body(nc, tc, ident, bias_t, fT, Wr, out_u, out_fT)
    nc.compile()
    return nc


def _emit_body(nc, tc, ident, bias_t, fT, Wr, out_u, out_fT):
    mm = nc.tensor.matmul

    with ExitStack() as body:
        # resident activations (bf16): qT/kT as [P, Cd*T] ([d,t] layout),
        # v as [P, Ct*D] ([k,e] layout)
        qtp = body.enter_context(tc.tile_pool(name="qt", bufs=3))
        ktp = body.enter_context(tc.tile_pool(name="kt", bufs=3))
        vtp = body.enter_context(tc.tile_pool(name="vt", bufs=3))
        qsp = body.enter_context(tc.tile_pool(name="qs", bufs=4))
        VDT = F8 if AV_FP8 else BF
        qT = [qtp.tile([P, Cd * T], BF, tag="qt", name=f"qT{i}") for i in range(3)]
        kT = [ktp.tile([P, Cd * T], BF, tag="kt", name=f"kT{i}") for i in range(3)]
        vv = [vtp.tile([P, Ct * D], VDT, tag="vt", name=f"v{i}") for i in range(3)]
        qs_tiles = {}

        def emit_qsums(qb):
            # qs23 = q2+q3 (for s1), qs12 = q1+q2 (for s3), one q-block each
            for key, (a, b) in (("23", (1, 2)), ("12", (0, 1))):
                t = qsp.tile([P, Cd * P], BF, tag="qs", name=f"qs{key}")
                nc.vector.tensor_tensor(
                    t[:].rearrange("p (c j) -> p c j", c=Cd),
                    qT[a][:].rearrange("p (c t) -> p c t", c=Cd)
                        [:, :, qb * P:(qb + 1) * P],
                    qT[b][:].rearrange("p (c t) -> p c t", c=Cd)
                        [:, :, qb * P:(qb + 1) * P],
                    mybir.AluOpType.add)
                qs_tiles[(key, qb)] = t

        # ---------------- Stage A: projections, all QKV -> SBUF ----------
        with ExitStack() as sA:
            ftp = sA.enter_context(tc.tile_pool(name="ft", bufs=1))
            fptp = sA.enter_context(tc.tile_pool(name="fpt", bufs=1))
            wsp = sA.enter_context(tc.tile_pool(name="wstream", bufs=2))
            psA = sA.enter_context(tc.tile_pool(name="psA", bufs=6, space="PSUM"))

            NH = D // WCH          # weight chunks per matrix
            EL = WCH // P          # output 128-blocks per chunk

            def wload(nm, i, h):
                w = wsp.tile([P, Cd * WCH], BF, tag="w", name=f"w{nm}")
                nc.sync.dma_start(
                    w[:].rearrange("p (c e) -> p c e", c=Cd),
                    Wr[(nm, i)].rearrange("p (c e) -> p c e", c=Cd)
                        [:, :, h * WCH:(h + 1) * WCH])
                return w

            for i in range(3):
                ft = ftp.tile([P, Cd * T], BF, tag="ft", name="ft")
                for dc in range(Cd):
                    nc.sync.dma_start(ft[:, dc * T:(dc + 1) * T],
                                      fT[i][dc * P:(dc + 1) * P, :])
                fpt = fptp.tile([P, Cd * T], BF, tag="fpt", name="fpt")
                # fpT_i = relu(Wp f^T + b); bf16 copy streams straight out
                for h in range(NH):
                    w = wload("p", i, h)
                    for el in range(EL):
                        ec = h * EL + el
                        for tn in range(T // S):
                            ps = psA.tile([P, S], FP, tag="psA")
                            for dc in range(Cd):
                                mm(ps[:], w[:, dc * WCH + el * P:dc * WCH + (el + 1) * P],
                                   ft[:, dc * T + tn * S:dc * T + (tn + 1) * S],
                                   start=dc == 0, stop=dc == Cd - 1)
                            dst = fpt[:, ec * T + tn * S:ec * T + (tn + 1) * S]
                            nc.scalar.activation(
                                dst, ps[:], AF.Relu,
                                bias=bias_t[:, i * Cd + ec:i * Cd + ec + 1])
                            nc.gpsimd.dma_start(
                                out_fT[i * D + ec * P:i * D + (ec + 1) * P,
                                       tn * S:(tn + 1) * S], dst)
                # qT_i / kT_i ([d,t] layout, feature chunk on partitions)
                for nm, dstt in (("q", qT[i]), ("k", kT[i])):
                    for h in range(NH):
                        w = wload(nm, i, h)
                        for el in range(EL):
                            oc = h * EL + el
                            for tn in range(T // S):
                                ps = psA.tile([P, S], FP, tag="psA")
                                for dc in range(Cd):
                                    mm(ps[:],
                                       w[:, dc * WCH + el * P:dc * WCH + (el + 1) * P],
                                       fpt[:, dc * T + tn * S:dc * T + (tn + 1) * S],
                                       start=dc == 0, stop=dc == Cd - 1)
                                nc.vector.tensor_copy(
                                    dstt[:, oc * T + tn * S:oc * T + (tn + 1) * S],
                                    ps[:])
                    if i == 2 and nm == "q":
                        emit_qsums(0)   # overlap first q-sums with Wk2/Wv2
                # v_i natural ([t,e] layout, t chunk on partitions);
                # bf16 matmul for accuracy, stored fp8 for the DR AV matmul
                for h in range(NH):
                    w = wload("v", i, h)
                    for tb in range(Ct):
                        ps = psA.tile([P, S], FP, tag="psA")
                        for dc in range(Cd):
                            mm(ps[:],
                               fpt[:, dc * T + tb * P:dc * T + (tb + 1) * P],
                               w[:, dc * WCH:(dc + 1) * WCH],
                               start=dc == 0, stop=dc == Cd - 1)
                        nc.scalar.copy(
                            vv[i][:, tb * D + h * WCH:tb * D + (h + 1) * WCH], ps[:])

        # ---------------- Stage B: scores + softmax + AV -----------------
        with ExitStack() as sB:
            ppp = sB.enter_context(tc.tile_pool(name="pp", bufs=3))
            ptsp = sB.enter_context(tc.tile_pool(name="pts", bufs=3))
            stp = sB.enter_context(tc.tile_pool(name="stats", bufs=2))
            usp = sB.enter_context(tc.tile_pool(name="us", bufs=4))
            psS = sB.enter_context(tc.tile_pool(name="psS", bufs=2, space="PSUM"))
            psU = sB.enter_context(tc.tile_pool(name="psU", bufs=2, space="PSUM"))
            psT = sB.enter_context(tc.tile_pool(name="psT", bufs=2, space="PSUM"))

            for qb in range(Ct):
                if qb + 1 < Ct:
                    emit_qsums(qb + 1)      # vector work for next block
                stats = stp.tile([P, 16], FP, tag="stats")
                p_t = [None] * 3
                pts_t = [None] * 3

                def softmax(s_ps, im):
                    negmax = stats[:, im:im + 1]
                    denom = stats[:, im + 4:im + 5]
                    recip = stats[:, im + 8:im + 9]
                    nc.vector.tensor_reduce(negmax, s_ps[:], mybir.AxisListType.X,
                                            mybir.AluOpType.max, negate=True)
                    p = ppp.tile([P, T], BF, tag="p", name="p")
                    nc.scalar.activation(p[:], s_ps[:], AF.Exp,
                                         bias=negmax, accum_out=denom)
                    nc.vector.reciprocal(recip, denom)
                    p_t[im] = p
                    pts_t[im] = ptsp.tile([P, Ct * P], VDT, tag="pts", name="pts")

                def transpose_chunk(im, kc):
                    pt = psT.tile([P, P], BF, tag="psT", name="pt")
                    nc.tensor.transpose(pt[:], p_t[im][:, kc * P:(kc + 1) * P],
                                        ident[:])
                    nc.vector.tensor_copy(
                        pts_t[im][:, kc * P:(kc + 1) * P], pt[:])

                def score(s_ps, prods):
                    # prods: list of (lhsT_tile, lhsT_col_fn, kT_tile)
                    for kb in range(T // S):
                        n = len(prods)
                        for j, (lt, colf, kt) in enumerate(prods):
                            for dc in range(Cd):
                                mm(s_ps[:, kb * S:(kb + 1) * S],
                                   lt[:, colf(dc):colf(dc) + P],
                                   kt[:, dc * T + kb * S:dc * T + (kb + 1) * S],
                                   start=(j == 0 and dc == 0),
                                   stop=(j == n - 1 and dc == Cd - 1))

                def av(im, interleave_im=None):
                    recip = stats[:, im + 8:im + 9]
                    pts3 = pts_t[im][:].rearrange("p (c j) -> p c j", c=Ct)
                    vv3 = vv[im][:].rearrange("p (c e) -> p c e", c=Ct)
                    for en in range(D // S):
                        psu = psU.tile([P, S], FP, tag="psU")
                        if AV_FP8:
                            for j, kc in enumerate(range(0, Ct, 2)):
                                mm(psu[:], pts3[:, kc:kc + 2, :],
                                   vv3[:, kc:kc + 2, en * S:(en + 1) * S],
                                   start=kc == 0, stop=kc == Ct - 2,
                                   perf_mode=DR)
                                if interleave_im is not None:
                                    transpose_chunk(interleave_im,
                                                    en * (Ct // 2) + j)
                        else:
                            for kc in range(Ct):
                                mm(psu[:], pts_t[im][:, kc * P:(kc + 1) * P],
                                   vv[im][:, kc * D + en * S:kc * D + (en + 1) * S],
                                   start=kc == 0, stop=kc == Ct - 1)
                                if interleave_im is not None and kc % 2 == 1:
                                    transpose_chunk(interleave_im, en * (Ct // 2)
                                                    + kc // 2)
                        us = usp.tile([P, S], BF, tag="us")
                        nc.vector.tensor_scalar_mul(us[:], psu[:], recip)
                        nc.gpsimd.dma_start(
                            out_u[qb * P:(qb + 1) * P,
                                  im * D + en * S:im * D + (en + 1) * S], us[:])

                qs23 = qs_tiles.pop(("23", qb))
                qs12 = qs_tiles.pop(("12", qb))
                qcol = lambda dc: dc * T + qb * P

                s1 = psS.tile([P, T], FP, tag="psS", name="s1")
                score(s1, [(qs23, lambda dc: dc * P, kT[0])])
                softmax(s1, 0)
                s2 = psS.tile([P, T], FP, tag="psS", name="s2")
                score(s2, [(qT[0], qcol, kT[2]), (qT[2], qcol, kT[1])])
                softmax(s2, 1)
                s3 = psS.tile([P, T], FP, tag="psS", name="s3")
                # interleave p1 transposes between s3 matmuls
                for kb in range(T // S):
                    for dc in range(Cd):
                        mm(s3[:, kb * S:(kb + 1) * S],
                           qs12[:, dc * P:(dc + 1) * P],
                           kT[2][:, dc * T + kb * S:dc * T + (kb + 1) * S],
                           start=dc == 0, stop=dc == Cd - 1)
                        if dc % 2 == 1:
                            transpose_chunk(0, kb * (Cd // 2) + dc // 2)
                softmax(s3, 2)
                av(0, interleave_im=1)
                av(1, interleave_im=2)
                av(2)


# ---------------------------------------------------------------------------
# Host side: runner + kernel()
# ---------------------------------------------------------------------------

def _make_runner(nc, n_cores=8):
    import jax
    from jax.sharding import Mesh, PartitionSpec
    from jax.experimental.shard_map import shard_map
    from concourse import mybir
    from concourse.bass2jax import (_bass_exec_p, install_neuronx_cc_hook,
                                    partition_id_tensor)

    install_neuronx_cc_hook()
    partition_name = (nc.partition_id_tensor.name
                      if nc.partition_id_tensor else None)
    in_names, out_names, out_avals, zero_outs = [], [], [], []
    for alloc in nc.m.functions[0].allocations:
        if not isinstance(alloc, mybir.MemoryLocationSet):
            continue
        name = alloc.memorylocations[0].name
        if alloc.kind == "ExternalInput":
            if name != partition_name:
                in_names.append(name)
        elif alloc.kind == "ExternalOutput":
            out_names.append(name)
            shape = tuple(alloc.tensor_shape)
            dtype = mybir.dt.np(alloc.dtype)
            out_avals.append(jax.core.ShapedArray(shape, dtype))
            zero_outs.append(np.zeros(shape, dtype))
    n_params = len(in_names)
    all_names = in_names + out_names
    if partition_name is not None:
        all_names.append(partition_name)

    def _body(*args):
        operands = list(args)
        if partition_name is not None:
            operands.append(partition_id_tensor())
        outs = _bass_exec_p.bind(
            *operands,
            out_avals=tuple(out_avals),
            in_names=tuple(all_names),
            out_names=tuple(out_names),
            lowering_input_output_aliases=(),
            sim_require_finite=True,
            sim_require_nnan=True,
            nc=nc,
        )
        return tuple(outs)

    devices = jax.devices()[:n_cores]
    mesh = Mesh(np.asarray(devices), ("core",))
    specs = (PartitionSpec("core"),)
    sharded = jax.jit(
        shard_map(_body, mesh=mesh,
                  in_specs=specs * (n_params + len(out_names)),
                  out_specs=specs * len(out_names), check_rep=False),
        keep_unused=True,
    )
    sharding = jax.sharding.NamedSharding(mesh, PartitionSpec("core"))

    def prepare(in_maps):
        per_core = [[np.asarray(m[name]) for name in in_names] for m in in_maps]
        concat_in = [np.concatenate([per_core[c][i] for c in range(n_cores)],
                                    axis=0) for i in range(n_params)]
        concat_zeros = [np.zeros((n_cores * z.shape[0], *z.shape[1:]), z.dtype)
                        for z in zero_outs]
        dev_in = [jax.device_put(a, sharding) for a in concat_in]
        dev_zero = [jax.device_put(a, sharding) for a in concat_zeros]
        jax.block_until_ready(dev_in)
        jax.block_until_ready(dev_zero)

        def execute():
            out = sharded(*dev_in, *dev_zero)
            jax.block_until_ready(out)
            return out

        execute.args = dev_in + dev_zero

        def fetch(out):
            return [
                {name: np.asarray(out[i]).reshape(n_cores, *out_avals[i].shape)[c]
                 for i, name in enumerate(out_names)}
                for c in range(n_cores)
            ]

        return execute, fetch

    def run(in_maps):
        execute, fetch = prepare(in_maps)
        return fetch(execute())

    run.prepare = prepare
    run.sharded = sharded
    return run


def _swizzle(W, dt):
    # [P, Cd*D] with (p, dc*D + o) = W[o, dc*128 + p]
    return np.ascontiguousarray(
        np.asarray(W).T.reshape(Cd, P, D).transpose(1, 0, 2).reshape(P, Cd * D)
    ).astype(dt)


def _prep_in_maps(inputs):
    import ml_dtypes
    shared = {}
    for i in range(3):
        for nm, key in (("p", "Wp"), ("q", "Wq"), ("k", "Wk"), ("v", "Wv")):
            shared[f"W{nm}r{i}"] = _swizzle(inputs[f"{key}{i+1}"],
                                            ml_dtypes.bfloat16)
        shared[f"bp{i}"] = np.asarray(inputs[f"bp{i+1}"], dtype=np.float32)
    f = [np.asarray(inputs[f"f{i+1}"]) for i in range(3)]
    in_maps = []
    for c in range(B):
        m = dict(shared)
        for i in range(3):
            m[f"fT{i}"] = np.ascontiguousarray(f[i][c].T).astype(
                ml_dtypes.bfloat16)
        in_maps.append(m)
    return in_maps


def get_runner(reps=1):
    if reps not in _STATE:
        nc = build_nc(reps=reps)
        _STATE[reps] = _make_runner(nc)
    return _STATE[reps]


def kernel(**inputs):
    run = get_runner()
    in_maps = _prep_in_maps(inputs)
    results = run(in_maps)
    out = np.empty((B, T, 6 * D), np.float32)
    for c in range(B):
        out[c, :, :3 * D] = results[c]["out_u"].astype(np.float32)
        out[c, :, 3 * D:] = results[c]["out_fT"].astype(np.float32).T
    return out
